# revision 10
# baseline (speedup 1.0000x reference)
"""Trainium2 Bass kernel for nn_EdgeEncoder (moe_routing).

Strategy
--------
Each of E edges is routed to 1 of 9 expert MLPs (4 -> 256 -> 256), then
  out = relu(concat([type_embed[tid], source_embed[sid], pv]) @ Wf + bf).

Host (numpy, cheap O(E) work):
  * scale/mask params, group edge indices by expert (base type),
  * split every expert's edges evenly over the 8 cores, padding each
    per-core expert segment to a multiple of 128 edges so all cores run
    ONE identical program (segment boundaries are compile-time constants),
  * fold biases into matmuls: x gets a ones-row (b1), the one-hot
    type/source rows get a ones-row whose G-row carries  b2@Wf_pv + bf,
  * precompute G = [type_embed @ Wf_t ; source_embed @ Wf_s ; c_t] so the
    embedding gathers become one K=20 matmul per 128 edges.

Device (per 512-edge block, edges grouped by expert):
  hT  = relu(W1e[t].T @ xT1)          2 matmuls  K=5   N=512  (features x edges)
  yT  = W2[t].T-chunks @ hT           4 matmuls  K=128 N=512
  out = uT.T@G_t + yT.T-chunks@Wf_pv  12 matmuls K=20/128 N=256 (edges x features)
  relu+copy PSUM->SBUF, DMA out rows (contiguous 128KB chunks).

Matmuls run as float32r (1 cycle/row on TRN2 for N>=256 vs 4 for fp32).
Set EDGEENC_MM_DT=float32 to force full-precision matmuls.
"""

import math
import os

import numpy as np

import concourse.bacc as bacc
import concourse.bass as bass
import concourse.mybir as mybir
import concourse.tile as tile
from concourse.bass_utils import run_bass_kernel_spmd

# ---- static module configuration (mirrors the torch source) ----
T = 9            # base types ("experts")
P_MAX = 4
D = 256
N_TYPES = 14
N_SRC = 5
NCORES = 8
BLOCK = 512      # edges per device block (one PSUM bank of fp32)
GRP = 128        # edge group granularity (PE partition dim)

BASE_MAP = np.array([0, 0, 0, 1, 1, 1, 2, 2, 3, 4, 5, 6, 7, 8], dtype=np.int32)
PCOUNT = np.array([2, 2, 1, 1, 1, 1, 3, 2, 4], dtype=np.int32)
SCALES = np.ones((T, P_MAX), dtype=np.float32)
SCALES[0, :2] = [1.0, 1e-06]      # nmos  m, w
SCALES[1, :2] = [1.0, 1e-06]      # pmos  m, w
SCALES[2, 0] = 1.0                # balun rout
SCALES[3, 0] = 1000.0             # resistor r
SCALES[4, 0] = 1e-12              # capacitor c
SCALES[5, 0] = 1e-09              # inductor l
SCALES[6, :3] = [1.0, 1.0, 1.0]   # vsource dc, mag, phase
SCALES[7, :2] = [0.001, 0.001]    # isource dc, mag
SCALES[8, :4] = [1.0, 1.0, 1e9, 1.0]  # port dbm, dc, freq, num

KX = 5                           # x rows: xT(4) + ones
KU = N_TYPES + N_SRC + 1          # 20 rows: type/source one-hot + ones

_MM_DT = (mybir.dt.float32 if os.environ.get("EDGEENC_MM_DT") == "float32"
          else mybir.dt.float32r)
_F32 = mybir.dt.float32
_BF16 = mybir.dt.bfloat16
# outT final layer: fewer, larger matmuls; output lands as [D, L] per core
_FINAL_OUTT = os.environ.get("EDGEENC_FINAL", "outT") == "outT"
# dense bf16 warm-up burst + per-block keep-warm matmuls: the PE HAM
# clock gate never un-throttles on fp32-HIGH-mode activity alone, so a
# kernel of pure f32r matmuls runs at 1.2 GHz instead of 2.4 GHz.
_WARM_BURST = int(os.environ.get("EDGEENC_WARM_BURST", "24"))
_WARM_EVERY = int(os.environ.get("EDGEENC_WARM_EVERY", "1"))

_PROGRAM_CACHE: dict = {}
LAST_RESULT = None  # BassKernelResults of the most recent run (for test harness)


def _layout(base_ids: np.ndarray):
    """Per-expert per-core segment sizes (multiples of GRP), identical on
    every core so one program serves all 8."""
    E = base_ids.shape[0]
    n_t = np.bincount(base_ids, minlength=T)
    m_t = np.zeros(T, dtype=np.int64)
    for t in range(T):
        if n_t[t] > 0:
            per_core = math.ceil(n_t[t] / NCORES)
            m_t[t] = math.ceil(per_core / GRP) * GRP
    L0 = int(m_t.sum())
    L = math.ceil(L0 / BLOCK) * BLOCK
    # fold the tail pad into the last present expert's segment
    last = int(np.nonzero(m_t)[0][-1])
    m_t[last] += L - L0
    return n_t, m_t, L


def _group_experts(m_t: np.ndarray) -> np.ndarray:
    """expert id of each 128-edge group, concatenated per expert."""
    return np.repeat(np.arange(T), (m_t // GRP))


def _build_order(base_ids: np.ndarray, n_t, m_t, L) -> np.ndarray:
    """ORD[c, j] = global edge index at per-core slot j (or -1 = pad)."""
    ORD = np.full((NCORES, L), -1, dtype=np.int64)
    off = 0
    for t in range(T):
        if m_t[t] == 0:
            continue
        seg = int(m_t[t])
        idx = np.nonzero(base_ids == t)[0]
        arr = np.full(NCORES * seg, -1, dtype=np.int64)
        arr[: idx.shape[0]] = idx
        ORD[:, off : off + seg] = arr.reshape(NCORES, seg)
        off += seg
    return ORD


def _host_inputs(type_ids, source_ids, params, ORD):
    """INX[c] = [5, L]: xT (scaled/masked) + ones row.
    INU[c] = [20, L]: type one-hot, source one-hot, ones row."""
    base_ids = BASE_MAP[type_ids]
    scales = SCALES[base_ids]                                  # [E,4]
    validp = np.arange(P_MAX)[None, :] < PCOUNT[base_ids][:, None]
    x = np.where(validp, params.astype(np.float32) / scales, 0.0).astype(np.float32)

    L = ORD.shape[1]
    INX = np.zeros((NCORES, KX, L), dtype=np.float32)
    INU = np.zeros((NCORES, KU, L), dtype=np.float32)
    valid = ORD >= 0
    ids = ORD[valid]
    tmp = np.zeros((NCORES, L, P_MAX), dtype=np.float32)
    tmp[valid] = x[ids]
    INX[:, 0:P_MAX, :] = tmp.transpose(0, 2, 1)
    INX[:, P_MAX, :] = valid
    ci, co = np.nonzero(valid)
    INU[ci, type_ids[ids], co] = 1.0
    INU[ci, N_TYPES + source_ids[ids], co] = 1.0
    INU[:, KU - 1, :] = valid
    return INX, INU


def _host_weights(type_embed, source_embed, W1, b1, W2, b2, Wf, bf):
    f = np.float32
    W1 = W1.astype(f); b1 = b1.astype(f); W2 = W2.astype(f); b2 = b2.astype(f)
    Wf = Wf.astype(f); bf = bf.astype(f)
    type_embed = type_embed.astype(f); source_embed = source_embed.astype(f)

    # layer1 lhsT blocks: [5, 9*256]; block t at cols [t*256,(t+1)*256)
    W1e = np.concatenate([W1, b1[:, None, :]], axis=1)          # [9,5,256]
    W1E = np.ascontiguousarray(W1e.transpose(1, 0, 2).reshape(5, T * D))

    # layer2 lhsT blocks: [128, 18*256]; block (t,h) = W2[t][h*128:(h+1)*128,:]
    W2R = np.ascontiguousarray(
        W2.reshape(T, 2, 128, D).transpose(2, 0, 1, 3).reshape(128, T * 2 * D)
    )

    # G_t [20,256]: type rows, source rows, const row (b2@Wf_pv + bf)
    Wft, Wfs, Wfp = Wf[:D], Wf[D : 2 * D], Wf[2 * D :]
    gt = type_embed @ Wft                                       # [14,256]
    gs = source_embed @ Wfs                                     # [5,256]
    gc = b2 @ Wfp + bf[None, :]                                 # [9,256]
    G = np.stack([np.concatenate([gt, gs, gc[t : t + 1]], axis=0) for t in range(T)])
    GSB = np.ascontiguousarray(G.transpose(1, 0, 2).reshape(N_TYPES + N_SRC + 1, T * D))

    # final pv lhsT... rhs blocks: [128, 2*256]; block h = Wf_pv[h*128:(h+1)*128,:]
    WFP = np.ascontiguousarray(
        Wfp.reshape(2, 128, D).transpose(1, 0, 2).reshape(128, 2 * D)
    )
    return W1E, W2R, GSB, WFP


def _build_program(m_t: tuple, L: int):
    """One compiled SPMD program for the given segment layout."""
    key = (m_t, L, str(_MM_DT), _FINAL_OUTT, _WARM_BURST, _WARM_EVERY)
    if key in _PROGRAM_CACHE:
        return _PROGRAM_CACHE[key]

    group_expert = _group_experts(np.asarray(m_t, dtype=np.int64))
    NB = L // BLOCK
    GP = BLOCK // GRP  # groups per block = 4

    nc = bacc.Bacc("TRN2", target_bir_lowering=False, debug=False,
                   num_devices=NCORES)
    inx_d = nc.dram_tensor("inx", [KX, L], _MM_DT, kind="ExternalInput")
    inu_d = nc.dram_tensor("inu", [KU, L], _MM_DT, kind="ExternalInput")
    w1e_d = nc.dram_tensor("w1e", [5, T * D], _MM_DT, kind="ExternalInput")
    w2r_d = nc.dram_tensor("w2r", [128, T * 2 * D], _MM_DT, kind="ExternalInput")
    g_d = nc.dram_tensor("gsb", [N_TYPES + N_SRC + 1, T * D], _MM_DT,
                         kind="ExternalInput")
    wfp_d = nc.dram_tensor("wfp", [128, 2 * D], _MM_DT, kind="ExternalInput")
    out_shape = [D, L] if _FINAL_OUTT else [L, D]
    out_d = nc.dram_tensor("out", out_shape, _F32, kind="ExternalOutput")

    RELU = mybir.ActivationFunctionType.Relu

    with tile.TileContext(nc) as tc:
        with (
            tc.tile_pool(name="wts", bufs=1) as wts,
            tc.tile_pool(name="inp", bufs=6) as inp,
            tc.tile_pool(name="hsb", bufs=4) as hsbp,
            tc.tile_pool(name="ysb", bufs=4) as ysbp,
            tc.tile_pool(name="osb", bufs=4) as osbp,
            tc.tile_pool(name="hps", bufs=4, space=bass.MemorySpace.PSUM) as hps,
            tc.tile_pool(name="yps", bufs=2, space=bass.MemorySpace.PSUM) as yps,
            tc.tile_pool(name="ops", bufs=2, space=bass.MemorySpace.PSUM) as ops,
        ):
            w1e = wts.tile([5, T * D], _MM_DT)
            w2r = wts.tile([128, T * 2 * D], _MM_DT)
            gsb = wts.tile([N_TYPES + N_SRC + 1, T * D], _MM_DT)
            wfp = wts.tile([128, 2 * D], _MM_DT)
            nc.gpsimd.dma_start(w1e[:], w1e_d.ap())
            nc.gpsimd.dma_start(w2r[:], w2r_d.ap())
            nc.gpsimd.dma_start(gsb[:], g_d.ap())
            nc.gpsimd.dma_start(wfp[:], wfp_d.ap())

            # bf16 scratch operands for HAM warm-up / keep-warm matmuls
            wmw = wts.tile([128, 128], _BF16)
            wma = wts.tile([128, BLOCK], _BF16)
            nc.vector.memset(wmw[:], 0.0)
            nc.vector.memset(wma[:], 0.0)
            if _WARM_BURST:
                wmp = hps.tile([GRP, BLOCK], _F32, name="warmps", tag="hts")
                for i in range(_WARM_BURST):
                    nc.tensor.matmul(wmp[:], wmw[:], wma[:], start=True,
                                     stop=True)

            for b in range(NB):
                g0 = b * GP
                experts = [int(group_expert[g0 + i]) for i in range(GP)]
                # runs of equal expert: (t, col0, col1) relative to block
                runs = []
                for i, t in enumerate(experts):
                    if runs and runs[-1][0] == t:
                        runs[-1] = (t, runs[-1][1], (i + 1) * GRP)
                    else:
                        runs.append((t, i * GRP, (i + 1) * GRP))

                xt_t = inp.tile([KX, BLOCK], _MM_DT, name=f"xt{b}", tag="xt")
                ut_t = inp.tile([KU, BLOCK], _MM_DT, name=f"ut{b}", tag="ut")
                nc.sync.dma_start(xt_t[:], inx_d.ap()[:, b * BLOCK : (b + 1) * BLOCK])
                nc.sync.dma_start(ut_t[:], inu_d.ap()[:, b * BLOCK : (b + 1) * BLOCK])
                xT = xt_t[:]
                uT = ut_t[:]

                # ---- layer 1: hT[h] = relu(W1e[t].T @ xT1) ----
                hts = [hps.tile([GRP, BLOCK], _F32, name=f"hts{b}_{j}", tag="hts") for j in range(2)]
                if _WARM_EVERY and b % _WARM_EVERY == 0:
                    # bf16 keep-warm matmul; layer-1's start=True overwrites it.
                    # fp32-HIGH activity is discounted by the PE HAM, so pure
                    # f32r streams re-throttle to 1.2 GHz after ~20us; a few
                    # percent of bf16 rows in the stream holds K=8/8.
                    nc.tensor.matmul(hts[0][:, 0:128], wmw[:], wma[:, 0:128],
                                     start=True, stop=True)
                for h in range(2):
                    for (t, c0, c1) in runs:
                        nc.tensor.matmul(
                            hts[h][:, c0:c1],
                            w1e[:, t * D + h * GRP : t * D + (h + 1) * GRP],
                            xT[:, c0:c1],
                            start=True, stop=True,
                        )
                hsb = [hsbp.tile([GRP, BLOCK], _MM_DT, name=f"hsb{b}_{j}", tag="hsb") for j in range(2)]
                nc.scalar.activation(hsb[0][:], hts[0][:], RELU)
                nc.vector.tensor_scalar_max(hsb[1][:], hts[1][:], 0.0)

                # ---- layer 2: yT[g] = sum_h W2[t][h].T-chunk @ hT[h] ----
                yts = [yps.tile([GRP, BLOCK], _F32, name=f"yts{b}_{j}", tag="yts") for j in range(2)]
                for g in range(2):
                    for (t, c0, c1) in runs:
                        for h in range(2):
                            nc.tensor.matmul(
                                yts[g][:, c0:c1],
                                w2r[:, (t * 2 + h) * D + g * GRP
                                    : (t * 2 + h) * D + (g + 1) * GRP],
                                hsb[h][:, c0:c1],
                                start=(h == 0), stop=(h == 1),
                            )
                ysb = [ysbp.tile([GRP, BLOCK], _MM_DT, name=f"ysb{b}_{j}", tag="ysb") for j in range(2)]
                nc.vector.tensor_copy(ysb[0][:], yts[0][:])
                nc.scalar.copy(ysb[1][:], yts[1][:])

                if _FINAL_OUTT:
                    # outT orientation: outT[n, e] per n-half; all N=BLOCK.
                    ots = [ops.tile([GRP, BLOCK], _F32, name=f"ots{b}_{j}", tag="ots") for j in range(2)]
                    if _WARM_EVERY and b % _WARM_EVERY == 0:
                        nc.tensor.matmul(ots[0][:, 0:128], wmw[:], wma[:, 0:128],
                                         start=True, stop=True)
                    for g in range(2):       # n-half
                        # one start/stop group per bank: Wfp chunk 0 opens
                        # (full-tile overwrite), G runs accumulate, chunk 1 closes
                        nc.tensor.matmul(
                            ots[g][:],
                            wfp[:, 0 * D + g * GRP : 0 * D + (g + 1) * GRP],
                            ysb[0][:],
                            start=True, stop=False,
                        )
                        for (t, c0, c1) in runs:
                            nc.tensor.matmul(
                                ots[g][:, c0:c1],
                                gsb[:, t * D + g * GRP : t * D + (g + 1) * GRP],
                                uT[:, c0:c1],
                                start=False, stop=False,
                            )
                        nc.tensor.matmul(
                            ots[g][:],
                            wfp[:, 1 * D + g * GRP : 1 * D + (g + 1) * GRP],
                            ysb[1][:],
                            start=False, stop=True,
                        )
                    osb = [osbp.tile([GRP, BLOCK], _F32, name=f"osb{b}_{j}", tag="osb") for j in range(2)]
                    nc.vector.tensor_scalar_max(osb[0][:], ots[0][:], 0.0)
                    nc.scalar.activation(osb[1][:], ots[1][:], RELU)
                    for g in range(2):
                        nc.sync.dma_start(
                            out_d.ap()[g * GRP : (g + 1) * GRP,
                                       b * BLOCK : (b + 1) * BLOCK],
                            osb[g][:],
                        )
                else:
                    # ---- final: out[e,:] = uT.T@G_t + sum_h yT[h].T-chunk@Wfp[h] ----
                    ots = [ops.tile([GRP, BLOCK], _F32, name=f"ots{b}_{j}", tag="ots") for j in range(2)]
                    for i in range(GP):          # 128-edge group, expert-pure
                        t = experts[i]
                        e0 = i * GRP
                        o = ots[i // 2][:, (i % 2) * D : (i % 2 + 1) * D]
                        nc.tensor.matmul(
                            o, uT[:, e0 : e0 + GRP],
                            gsb[:, t * D : (t + 1) * D],
                            start=True, stop=False,
                        )
                        for h in range(2):
                            nc.tensor.matmul(
                                o, ysb[h][:, e0 : e0 + GRP],
                                wfp[:, h * D : (h + 1) * D],
                                start=False, stop=(h == 1),
                            )
                    osb = [osbp.tile([GRP, BLOCK], _F32, name=f"osb{b}_{j}", tag="osb") for j in range(2)]
                    nc.vector.tensor_scalar_max(osb[0][:], ots[0][:], 0.0)
                    nc.vector.tensor_scalar_max(osb[1][:], ots[1][:], 0.0)
                    for i in range(GP):
                        r0 = b * BLOCK + i * GRP
                        nc.sync.dma_start(
                            out_d.ap()[r0 : r0 + GRP, :],
                            osb[i // 2][:, (i % 2) * D : (i % 2 + 1) * D],
                        )

    nc.compile()
    _PROGRAM_CACHE[key] = nc
    return nc


def kernel(type_ids, source_ids, params, type_embed, source_embed,
           W1, b1, W2, b2, Wf, bf):
    global LAST_RESULT
    type_ids = np.asarray(type_ids, dtype=np.int32)
    source_ids = np.asarray(source_ids, dtype=np.int32)
    params = np.asarray(params, dtype=np.float32)
    E = type_ids.shape[0]

    base_ids = BASE_MAP[type_ids]
    n_t, m_t, L = _layout(base_ids)
    ORD = _build_order(base_ids, n_t, m_t, L)
    INX, INU = _host_inputs(type_ids, source_ids, params, ORD)
    W1E, W2R, GSB, WFP = _host_weights(
        np.asarray(type_embed), np.asarray(source_embed),
        np.asarray(W1), np.asarray(b1), np.asarray(W2), np.asarray(b2),
        np.asarray(Wf), np.asarray(bf))

    nc = _build_program(tuple(int(v) for v in m_t), L)

    in_maps = [
        {"inx": np.ascontiguousarray(INX[c]), "inu": np.ascontiguousarray(INU[c]),
         "w1e": W1E, "w2r": W2R, "gsb": GSB, "wfp": WFP}
        for c in range(NCORES)
    ]
    trace = bool(int(os.environ.get("EDGEENC_TRACE", "0")))
    res = run_bass_kernel_spmd(nc, in_maps, core_ids=list(range(NCORES)),
                               trace=trace)
    LAST_RESULT = res

    full = np.zeros((E, D), dtype=np.float32)
    for c in range(NCORES):
        sel = ORD[c] >= 0
        oc = res.results[c]["out"]
        if _FINAL_OUTT:
            full[ORD[c][sel]] = np.ascontiguousarray(oc[:, sel].T)
        else:
            full[ORD[c][sel]] = oc[sel]
    return full
